# revision 48
# baseline (speedup 1.0000x reference)
"""NetVLAD forward on 8 Trainium2 NeuronCores.

Reference computation (per batch b):
    logits = conv_w @ x_flat[b]            # [K, N]    (K=64, C=128, N=4096)
    a      = softmax(logits, axis=K)
    vlad   = a @ x_flat[b].T - sum_n(a) * centroids    # [K, C]
    out[b] = l2norm over C then over K*C  (epilogue on host)

Sharding: data-parallel over batch (8 per core); conv weight replicated.

v3 design (DMA-byte-bound: ~23.5us of input transfers at the modeled
360 B/ns):
  - x ships twice in fp8 (same bytes as one bf16 copy): x1 = [C, N] for
    mm1, x2 = [n%128, n//128, C+2] (cols: x | -1 | 0-pad) for mm2.
  - one whole-batch DMA per x1/x2 (fewer HWDGE slots); the last two
    batches' x2 arrive as fine slices so the mm2 tail after the final
    transfer is ~1 slice, not a whole batch.
  - softmax reductions all on DVE in bf16 (2x_1p mode); exp on ACT;
    GPSIMD (Pool) is left free to host the output path.
  - outputs leave via kv_writeback prepare/trigger on Pool: descriptors
    are generated off the critical path, and the data-dependent trigger
    skips the HWDGE(625ns)+DGE-delay(650ns) latency a plain store DMA
    pays after its wait. Out tile is [128, 130] (rows 64..127 unused pad
    required by the 128-partition writeback layout; host reads rows
    0..63 = [vlad_raw | -asum | pad]).
  - post-compile fixup: each kv prep's on_update[0] is retargeted to the
    DMASW queue sem the Tile epilogue waits on (the real ucode bumps it
    via ring bookkeeping; the no-exec timeline sim only fires the
    instruction's recorded update).
"""

import numpy as np
import ml_dtypes
from contextlib import ExitStack

import concourse.bass as bass
import concourse.bacc as bacc
import concourse.tile as tile
import concourse.mybir as mybir
from concourse import bass_utils

B, C, K = 64, 128, 64
HW = 64 * 64
NCORES = 8
BPC = B // NCORES
F32 = mybir.dt.float32
BF16 = mybir.dt.bfloat16
FP8 = mybir.dt.float8e4
I32 = mybir.dt.int32

NCHUNK = 128
CHUNKS = HW // NCHUNK     # 32
GROUP = 16                # chunks per softmax group
NG = CHUNKS // GROUP      # 2
CP2 = C + 2               # x2 row: [x(128) | -1 | 0]
OUTW = CP2                # psum/out tile width (130 = 2*65)

# ---- schedule (tunable) ----------------------------------------------------
# stream: DMA issue order. ("x1", b, [(lo,hi)...] in n-cols/128) or
# ("x2", b, [(lo,hi)...] in chunks) or ("w",)
_CFG = {
    "stream": [
        ("x1", 0, [(0, 16), (16, 32)]), ("w",),
        ("x1", 1, [(0, 32)]), ("x2", 0, [(0, 16), (16, 32)]),
        ("x1", 2, [(0, 32)]), ("x2", 1, [(0, 16), (16, 32)]),
        ("x1", 3, [(0, 32)]), ("x2", 2, [(0, 16), (16, 32)]),
        ("x1", 4, [(0, 32)]), ("x2", 3, [(0, 16), (16, 32)]),
        ("x1", 5, [(0, 32)]), ("x2", 4, [(0, 16), (16, 32)]),
        ("x1", 6, [(0, 32)]), ("x1", 7, [(0, 32)]),
        ("x2", 5, [(0, 16), (16, 32)]),
        ("x2", 6, [(0, 16), (16, 32)]),
        ("x2", 7, [(0, 16), (16, 32)]),
    ],
    # pe order: ("A", b[, groups]) = mm1, ("C", b, lo, hi) = mm2 chunks,
    # ("P", b) = psum->sbuf copy
    "pe": [
        ("A", 0), ("A", 1), ("C", 0, 0, 16), ("C", 0, 16, 32), ("A", 2),
        ("C", 1, 0, 16), ("C", 1, 16, 32), ("P", 0), ("A", 3),
        ("C", 2, 0, 16), ("C", 2, 16, 32), ("P", 1), ("A", 4),
        ("C", 3, 0, 16), ("C", 3, 16, 32), ("P", 2), ("A", 5),
        ("C", 4, 0, 16), ("C", 4, 16, 32), ("P", 3), ("A", 6), ("A", 7),
        ("C", 5, 0, 16), ("C", 5, 16, 32), ("P", 4),
        ("C", 6, 0, 16), ("C", 6, 16, 32), ("P", 5),
        ("C", 7, 0, 16), ("C", 7, 16, 32), ("P", 6), ("P", 7),
    ],
    "copy_pool": False,    # gpsimd PSUM->SBUF copy breaks the axon backend
    "sub": {},             # softmax chains per group (default 1 = 16-chunk)
    "copy7_pool": False,   # gpsimd PSUM copy unsupported on device
    "h_pool": set(),       # batches whose K/2-halving add runs on gpsimd
    "h2_pool": True,       # K/4-halving add on gpsimd (frees DVE pace)
    "red_pool": False,     # gpsimd can't do X-axis reduce
    "nwarm": 16,
}


_LABELS = {}


def _lab(inst, txt):
    try:
        _LABELS[inst.ins.name] = txt
    except Exception:
        pass
    return inst


def _netvlad_tile(tc: tile.TileContext, out_d, x1_d, x2_d, w_d, cfg):
    nc = tc.nc
    with ExitStack() as ctx:
        const = ctx.enter_context(tc.tile_pool(name="const", bufs=1))
        # input pools sized so no input DMA ever waits on a slot (a waiting
        # DMA blocks the in-order SP queue and lets the scheduler scramble
        # the stream order)
        x1pool = ctx.enter_context(tc.tile_pool(name="x1", bufs=7))
        x2pool = ctx.enter_context(tc.tile_pool(name="x2", bufs=12))
        epool = ctx.enter_context(tc.tile_pool(name="e", bufs=6))
        hpool = ctx.enter_context(tc.tile_pool(name="h", bufs=6))
        h2pool = ctx.enter_context(tc.tile_pool(name="h2", bufs=6))
        spool = ctx.enter_context(tc.tile_pool(name="s", bufs=8))
        apool = ctx.enter_context(tc.tile_pool(name="a", bufs=12))
        opool = ctx.enter_context(tc.tile_pool(name="o", bufs=1))
        pl_pool = ctx.enter_context(tc.tile_pool(name="pl", bufs=3, space="PSUM"))
        pv_pool = ctx.enter_context(tc.tile_pool(name="pv", bufs=2, space="PSUM"))

        w_sb = const.tile([C, K], BF16)
        idx_sb = const.tile([128, 1], I32)

        x1t = {}   # b -> list of (lo, hi, tile)  (n-col blocks /128)
        x2t = {}   # b -> list of (lo, hi, tile)  (chunk blocks)
        avst = {}  # b -> list of (lo, hi, a_tile) chunk ranges
        pvt = {}   # b -> psum tile
        outt = {}  # b -> sbuf out tile

        # w load first on the Pool queue: its desc-gen must precede the kv
        # prep generation or mm1 starves
        nc.gpsimd.dma_start(out=w_sb, in_=w_d)
        # one [128, 8*130] out tile: per-batch [64, 130] blocks land in rows
        # 0..63 of column block b; rows 64..127 are pad the 128-partition
        # kv_writeback source layout requires
        osb = opool.tile([128, BPC * OUTW], F32, tag="o")
        nc.gpsimd.memset(idx_sb, 0)
        nc.gpsimd.memset(osb, 0.0)

        def load_x1(b, blocks):
            x1t[b] = []
            for lo, hi in blocks:
                t = x1pool.tile([C, (hi - lo) * NCHUNK], FP8, tag=f"x1_{hi-lo}")
                _lab(nc.sync.dma_start(out=t, in_=x1_d[b][:, lo * NCHUNK : hi * NCHUNK]), f"dma_x1_{b}_{lo}")
                x1t[b].append((lo, hi, t))

        def load_x2(b, blocks):
            x2t.setdefault(b, [])
            for lo, hi in blocks:
                t = x2pool.tile([NCHUNK, hi - lo, CP2], FP8, tag=f"x2_{hi-lo}")
                _lab(nc.sync.dma_start(out=t, in_=x2_d[b][:, lo:hi, :]), f"dma_x2_{b}_{lo}")
                x2t[b].append((lo, hi, t))

        def x1_block(b, ch):
            # [C, 128] slice of x1 covering chunk ch
            for lo, hi, t in x1t[b]:
                if lo <= ch < hi:
                    return t[:, (ch - lo) * NCHUNK : (ch - lo + 1) * NCHUNK]
            raise KeyError((b, ch))

        def x2_block(b, ch):
            for lo, hi, t in x2t[b]:
                if lo <= ch < hi:
                    return t[:, ch - lo, :]
            raise KeyError((b, ch))

        def softmax_chain(pl, glo, lo, hi, tag, h_eng, h2_pool):
            """exp+normalize chunks [lo,hi) of a group whose psum tile pl
            starts at chunk glo. Returns the a tile (width hi-lo)."""
            w_ = hi - lo
            e = epool.tile([NCHUNK, K, w_], BF16, tag=f"e{w_}")
            e_gk = bass.AP(tensor=e.tensor, offset=e.offset,
                           ap=[e.ap[0], e.ap[2], e.ap[1]])
            _lab(nc.scalar.activation(e_gk, pl[:, lo - glo : hi - glo, :],
                                 mybir.ActivationFunctionType.Exp), f"exp_{tag}")
            h = hpool.tile([NCHUNK, K // 2, w_], BF16, tag=f"h{w_}")
            h2 = h2pool.tile([NCHUNK, K // 4, w_], BF16, tag=f"h2{w_}")
            with nc.allow_low_precision(reason="bf16 softmax partials; error averages over n"):
                _lab(h_eng.tensor_tensor(out=h, in0=e[:, 0 : K // 2, :],
                                    in1=e[:, K // 2 : K, :], op=mybir.AluOpType.add), f"h_{tag}")
                h2_eng = nc.gpsimd if h2_pool else nc.vector
                h2_eng.tensor_tensor(out=h2, in0=h[:, 0 : K // 4, :],
                                     in1=h[:, K // 4 : K // 2, :], op=mybir.AluOpType.add)
                h2_gk = bass.AP(tensor=h2.tensor, offset=h2.offset,
                                ap=[h2.ap[0], [1, w_], h2.ap[1]])
                s = spool.tile([NCHUNK, w_], BF16, tag=f"s{w_}")
                red_eng = nc.gpsimd if cfg.get("red_pool") else nc.vector
                red_eng.reduce_sum(s, h2_gk, axis=mybir.AxisListType.X)
                r = spool.tile([NCHUNK, w_], BF16, tag=f"r{w_}")
                nc.vector.reciprocal(r, s)
                a = apool.tile([NCHUNK, K, w_], BF16, tag=f"a{w_}")
                r_bh = bass.AP(tensor=r.tensor, offset=r.offset,
                               ap=[r.ap[0], [0, K], [1, w_]])
                _lab(nc.vector.tensor_tensor(out=a, in0=e, in1=r_bh, op=mybir.AluOpType.mult), f"scale_{tag}")
            return a

        def stage_A(b, gs=None):
            sub = cfg["sub"].get(b, 1)
            h_eng = nc.gpsimd if b in cfg["h_pool"] else nc.vector
            avst.setdefault(b, [])
            for g in (range(NG) if gs is None else gs):
                pl = pl_pool.tile([NCHUNK, GROUP, K], F32, tag="pl")
                for i in range(GROUP):
                    ch = g * GROUP + i
                    _lab(nc.tensor.matmul(pl[:, i, :], lhsT=x1_block(b, ch), rhs=w_sb,
                                     start=True, stop=True), f"mm1_b{b}_ch{ch}")
                w_ = GROUP // sub
                for q in range(sub):
                    lo = g * GROUP + q * w_
                    a = softmax_chain(pl, g * GROUP, lo, lo + w_, f"b{b}c{lo}", h_eng,
                                      b in cfg.get("h2_pool_b", set()))
                    avst[b].append((lo, lo + w_, a))

        def a_block(b, ch):
            for lo, hi, t in avst[b]:
                if lo <= ch < hi:
                    return t[:, :, ch - lo]
            raise KeyError((b, ch))

        def stage_C(b, lo, hi):
            if b not in pvt:
                pv_new = pv_pool.tile([K, OUTW], F32, tag="pv")
                pvt[b] = pv_new
            pv = pvt[b]
            for ch in range(lo, hi):
                _lab(nc.tensor.matmul(pv, lhsT=a_block(b, ch), rhs=x2_block(b, ch),
                                 start=(ch == 0), stop=(ch == CHUNKS - 1)), f"mm2_b{b}_ch{ch}")

        def stage_P(b):
            # copy psum -> sbuf, then kv prep + trigger (prep after producer
            # is the only ordering the real ucode path supports)
            pv = pvt[b]
            dst = osb[0:K, b * OUTW : (b + 1) * OUTW]
            _lab(nc.scalar.copy(out=dst, in_=pv), f"copy_{b}")
            out_ap = bass.AP(
                tensor=out_d.tensor, offset=out_d.offset + b * 128 * OUTW,
                ap=[[0, 1], [OUTW, 128], [OUTW // 2, 2], [1, OUTW // 2]])
            in_ap = bass.AP(
                tensor=osb.tensor, offset=osb.offset + b * OUTW,
                ap=[list(osb.ap[0]), [OUTW // 2, 2], [0, 1], [1, OUTW // 2]])
            nc.gpsimd.kv_writeback(out_ap, in_ap, idx_sb, prepare_only=True,
                                   sem=nc.alloc_semaphore(f"kvsem{b}"))
            _lab(nc.gpsimd.trigger_dma(count=None), f"trig_{b}")

        # PE p-state warmup
        warm_a = const.tile([NCHUNK, K], BF16)
        warm_b = const.tile([NCHUNK, OUTW], BF16)
        nc.vector.memset(warm_a, 0.0)
        nc.vector.memset(warm_b, 0.0)
        wpv = pv_pool.tile([K, OUTW], F32, tag="pv")
        NWARM = cfg["nwarm"]
        for i in range(NWARM):
            nc.tensor.matmul(wpv, lhsT=warm_a, rhs=warm_b,
                             start=(i == 0), stop=(i == NWARM - 1))
        warm_out = const.tile([K, 1], F32)
        nc.vector.tensor_copy(out=warm_out, in_=wpv[:, 0:1])

        # DMA stream (w already issued on the Pool queue)
        for ent in cfg["stream"]:
            if ent[0] == "w":
                continue
            elif ent[0] == "x1":
                load_x1(ent[1], ent[2])
            else:
                load_x2(ent[1], ent[2])

        # compute in arrival order
        for ent in cfg["pe"]:
            if ent[0] == "A":
                stage_A(ent[1], ent[2] if len(ent) > 2 else None)
            elif ent[0] == "P":
                stage_P(ent[1])
            else:
                _, b, lo, hi = ent
                stage_C(b, lo, hi)




def _fixup_kv_sems(nc):
    """Point each kv prep's recorded DMA-completion update at the DMASW
    queue sem the epilogue waits on (sim-side bookkeeping only; the real
    descriptor uses instr.sem_num and the HW ring bumps DMASW itself)."""
    from concourse.tile_sem_assignment import PROC_NAME_TO_IDX

    idx_to_lane = {v: k for k, v in PROC_NAME_TO_IDX.items() if k.startswith("DMASW")}
    fn = nc.m.functions[0]
    lane_sem = {}   # "DMASW3" -> sem id
    preps = []
    for bb in fn.blocks:
        for inst in bb.instructions:
            si = inst.sync_info
            if si is None:
                continue
            for w in list(si.on_wait) + list(si.on_update):
                nm = str(getattr(w, "ant_name", "") or "")
                if nm.startswith("DMASW"):
                    lane_sem[nm.split("_")[0]] = w.id
            if "KVWriteback" in str(inst.opcode):
                preps.append(inst)
    assert preps, "no kv preps found"
    for p in preps:
        lane = idx_to_lane[p.bass_scheduled_proc]
        p.sync_info.on_update[0].id = lane_sem[lane]


_NC_CACHE = None


def _get_nc():
    global _NC_CACHE
    if _NC_CACHE is None:
        nc = bacc.Bacc("TRN2", target_bir_lowering=False, debug=False,
                       num_devices=NCORES, dynamic_dma_scratch_size=65536)
        x1_d = nc.dram_tensor("x1", [BPC, C, HW], FP8, kind="ExternalInput").ap()
        x2_d = nc.dram_tensor("x2", [BPC, NCHUNK, CHUNKS, CP2], FP8,
                              kind="ExternalInput").ap()
        w_d = nc.dram_tensor("w_t", [C, K], BF16, kind="ExternalInput").ap()
        out_d = nc.dram_tensor("out", [BPC, 128, OUTW], F32,
                               kind="ExternalOutput").ap()
        with tile.TileContext(nc) as tc:
            _netvlad_tile(tc, out_d, x1_d, x2_d, w_d, _CFG)
        nc.compile()
        _fixup_kv_sems(nc)
        _NC_CACHE = nc
    return _NC_CACHE


def _make_in_maps(x, conv_w):
    bf16 = ml_dtypes.bfloat16
    f8 = ml_dtypes.float8_e4m3fn
    x1 = np.ascontiguousarray(x.reshape(B, C, HW)).astype(f8)
    xt = np.ascontiguousarray(x1.reshape(B, C, CHUNKS, NCHUNK).transpose(0, 3, 2, 1))
    x2 = np.empty((B, NCHUNK, CHUNKS, CP2), dtype=f8)
    x2[..., :C] = xt
    x2[..., C] = -1.0
    x2[..., C + 1] = 0.0
    w_t = np.ascontiguousarray(conv_w.T.astype(bf16))
    in_maps = []
    for core in range(NCORES):
        sl = slice(core * BPC, (core + 1) * BPC)
        in_maps.append({"x1": x1[sl], "x2": x2[sl], "w_t": w_t})
    return in_maps


def _run(in_maps, trace=False, **kwargs):
    nc = _get_nc()
    return bass_utils.run_bass_kernel_spmd(
        nc, in_maps, core_ids=list(range(NCORES)), trace=trace, **kwargs)


def _postprocess(raw, centroids):
    """raw: [B, 128, OUTW]; rows 0..63 = [vlad_raw | -asum | pad]."""
    vlad = raw[:, :K, :C] + raw[:, :K, C : C + 1] * centroids[None, :, :]
    norms = np.sqrt((vlad * vlad).sum(axis=2, keepdims=True))
    vlad = vlad / np.maximum(norms, 1e-12)
    out = vlad.reshape(raw.shape[0], K * C)
    gn = np.sqrt((out * out).sum(axis=1, keepdims=True))
    return out / np.maximum(gn, 1e-12)


def kernel(x, conv_w, centroids):
    x = np.asarray(x)
    conv_w = np.asarray(conv_w)
    centroids = np.asarray(centroids, dtype=np.float32)
    res = _run(_make_in_maps(x, conv_w))
    raw = np.concatenate([r["out"] for r in res.results], axis=0)
    return _postprocess(raw.astype(np.float32), centroids).astype(np.float32)


# revision 52
# speedup vs baseline: 1.1195x; 1.1195x over previous
"""NetVLAD forward on 8 Trainium2 NeuronCores.

Reference computation (per batch b):
    logits = conv_w @ x_flat[b]            # [K, N]    (1x1 conv, K=64, C=128, N=4096)
    a      = softmax(logits, axis=K)
    vlad   = a @ x_flat[b].T - sum_n(a) * centroids    # [K, C]
    vlad   = l2norm(vlad, axis=C)          # intra-normalize
    out[b] = l2norm(vlad.reshape(K*C))     # global normalize

Sharding: pure data-parallel over the batch dim (8 batches per core);
conv weight replicated.  No collectives needed.

v2 design (DMA-bound ~24us/core in the timeline model):
  - x is shipped to the device TWICE in fp8-e4m3 (same total bytes as one
    bf16 copy): x1 = [C, N] layout feeding mm1 (logits), and x2 = a
    host-pre-transposed [n%128, n//128, C+1] layout feeding mm2 directly,
    with a -1 column baked in for the -sum(a) term.  This removes the PE
    transpose AND the PSUM->SBUF copies of x^T that dominated v1.
  - mm1 runs mixed-dtype (fp8 x * bf16 w) so the tiny conv weight keeps
    full precision (w quantization error is systematic across n and does
    not average out; x quantization does).
  - softmax over k (free dim): ACT exp (batched 16 chunks); the k-sum as
    a GPSIMD half-add + DVE quarter-add + quarter-sized DVE reduce; the
    1/s scale as a DVE tensor_tensor in a [p, k, chunk] layout whose
    innermost dim is packed bf16 -> qualifies for the 2x_1p DVE perf mode.
  - a (bf16) @ x2 (fp8) accumulates [vlad_raw | -asum] in one PSUM bank
    per batch; tiny epilogue (centroid subtraction + two L2 norms) on the
    host, as in v1.
  - scheduling: PE p-state warmup, x1 loads lead x2 by ~2 batches (tuned
    against the timeline model: every softmax chain completes during the
    DMA stream; late x2s gate only PE mm2 work), compute emitted in
    stream-arrival order so softmax-gated mm2s never head-of-line block
    later batches' logits in the in-order PE queue.
"""

import numpy as np
import ml_dtypes
from contextlib import ExitStack

import concourse.bass as bass
import concourse.bacc as bacc
import concourse.tile as tile
import concourse.mybir as mybir
from concourse import bass_utils

B, C, K = 64, 128, 64
HW = 64 * 64  # N = H*W
NCORES = 8
BPC = B // NCORES  # batches per core
F32 = mybir.dt.float32
BF16 = mybir.dt.bfloat16
FP8 = mybir.dt.float8e4

NCHUNK = 128              # n-columns per chunk (PE partition limit)
CHUNKS = HW // NCHUNK     # 32 chunks per batch
GROUP = 16                # chunks per group (one ACT/DVE batch, 2 psum banks)
NG = CHUNKS // GROUP      # groups per batch = 2


def _netvlad_tile(tc: tile.TileContext, out_d, x1_d, x2_d, w_d):
    nc = tc.nc
    with ExitStack() as ctx:
        const = ctx.enter_context(tc.tile_pool(name="const", bufs=1))
        x1pool = ctx.enter_context(tc.tile_pool(name="x1", bufs=2 * NG * 4))
        x2pool = ctx.enter_context(tc.tile_pool(name="x2", bufs=2 * NG * 4))
        epool = ctx.enter_context(tc.tile_pool(name="e", bufs=3 * NG))
        hpool = ctx.enter_context(tc.tile_pool(name="h", bufs=3 * NG))
        h2pool = ctx.enter_context(tc.tile_pool(name="h2", bufs=3 * NG))
        apool = ctx.enter_context(tc.tile_pool(name="a", bufs=BPC * NG))
        spool = ctx.enter_context(tc.tile_pool(name="s", bufs=6 * NG))
        opool = ctx.enter_context(tc.tile_pool(name="o", bufs=BPC))
        pl_pool = ctx.enter_context(tc.tile_pool(name="pl", bufs=3, space="PSUM"))
        pv_pool = ctx.enter_context(tc.tile_pool(name="pv", bufs=2, space="PSUM"))

        w_sb = const.tile([C, K], BF16)

        outts = []
        x1t = {}  # ib -> [x1 tile per group]
        x2t = {}  # ib -> [x2 tile per group]
        avst = {}  # ib -> [a tile per group]

        def load_x1(ib, parts=1):
            # parts>1: finer first loads so batch 0's first exp starts earlier
            x1t[ib] = []
            pw = GROUP * NCHUNK // parts
            for g in range(NG):
                gparts = []
                for p in range(parts):
                    x1g = x1pool.tile([C, pw], FP8, tag=f"x1p{parts}")
                    nc.sync.dma_start(
                        out=x1g,
                        in_=x1_d[ib][
                            :,
                            g * GROUP * NCHUNK + p * pw : g * GROUP * NCHUNK + (p + 1) * pw,
                        ],
                    )
                    gparts.append(x1g)
                x1t[ib].append(gparts)

        def load_x2(ib):
            x2t[ib] = []
            for g in range(NG):
                x2g = x2pool.tile([NCHUNK, GROUP, C + 1], FP8, tag="x2")
                nc.sync.dma_start(
                    out=x2g, in_=x2_d[ib][:, g * GROUP : (g + 1) * GROUP, :]
                )
                x2t[ib].append(x2g)

        def softmax_chain(pl_slice, width, tag, pool_half, pool_h2=False, split_scale=1):
            """exp + normalize `width` chunks of logits; returns the a tile."""
            e = epool.tile([NCHUNK, K, width], BF16, tag=f"e{tag}")
            e_gk = bass.AP(
                tensor=e.tensor, offset=e.offset, ap=[e.ap[0], e.ap[2], e.ap[1]]
            )
            nc.scalar.activation(e_gk, pl_slice, mybir.ActivationFunctionType.Exp)

            # sum over k: GPSIMD (otherwise idle) halves, DVE (2x_1p mode)
            # quarters, then a quarter-sized DVE reduce
            h = hpool.tile([NCHUNK, K // 2, width], BF16, tag=f"h{tag}")
            h2 = h2pool.tile([NCHUNK, K // 4, width], BF16, tag=f"h2{tag}")
            with nc.allow_low_precision(reason="bf16 partial softmax sum; 0.4% on r averages out over n"):
                half_eng = nc.gpsimd if pool_half else nc.vector
                half_eng.tensor_tensor(
                    out=h,
                    in0=e[:, 0 : K // 2, :],
                    in1=e[:, K // 2 : K, :],
                    op=mybir.AluOpType.add,
                )
                (nc.gpsimd if pool_h2 else nc.vector).tensor_tensor(
                    out=h2,
                    in0=h[:, 0 : K // 4, :],
                    in1=h[:, K // 4 : K // 2, :],
                    op=mybir.AluOpType.add,
                )
            a = apool.tile([NCHUNK, K, width], BF16, tag=f"a{tag}")
            nsplit = split_scale
            hw_ = width // nsplit
            for q in range(nsplit):
                # with nsplit=2 (drain tail): the whole reduce/recip/scale
                # chain runs per half so the first half's mm2s start early
                h2_gk = bass.AP(
                    tensor=h2.tensor,
                    offset=h2.offset + q * hw_,
                    ap=[h2.ap[0], [1, hw_], h2.ap[1]],
                )
                s = spool.tile([NCHUNK, hw_], F32, tag=f"s{tag}q{nsplit}")
                nc.vector.reduce_sum(s, h2_gk, axis=mybir.AxisListType.X)
                r = spool.tile([NCHUNK, hw_], BF16, tag=f"r{tag}q{nsplit}")
                with nc.allow_low_precision(reason="bf16 r enables the 2x DVE mode on the scale; error averages out over n"):
                    nc.vector.reciprocal(r, s)
                r_bh = bass.AP(
                    tensor=r.tensor, offset=r.offset, ap=[r.ap[0], [0, K], [1, hw_]]
                )
                nc.vector.tensor_tensor(
                    out=a[:, :, q * hw_ : (q + 1) * hw_],
                    in0=e[:, :, q * hw_ : (q + 1) * hw_],
                    in1=r_bh,
                    op=mybir.AluOpType.mult,
                )
            return a

        def stage_AB(ib, sub=1):
            # mm1 logits for all groups, then the softmax chains.  sub>1
            # splits each group's softmax into sub slices so the tail chain
            # pipelines at finer granularity (used for the drain batches).
            pls = []
            parts = len(x1t[ib][0])
            ppg = GROUP // parts  # chunks per part
            for g in range(NG):
                pl = pl_pool.tile([NCHUNK, GROUP, K], F32, tag="pl")
                for i in range(GROUP):
                    xsrc = x1t[ib][g][i // ppg]
                    nc.tensor.matmul(
                        pl[:, i, :],
                        lhsT=xsrc[:, (i % ppg) * NCHUNK : (i % ppg + 1) * NCHUNK],
                        rhs=w_sb,
                        start=True,
                        stop=True,
                    )
                pls.append(pl)

            w_ = GROUP // sub
            avs = []
            for g in range(NG):
                for q in range(sub):
                    avs.append(
                        softmax_chain(
                            pls[g][:, q * w_ : (q + 1) * w_, :],
                            w_,
                            str(w_),
                            pool_half=True,
                            pool_h2=(ib == BPC - 1 and g == NG - 1),
                            split_scale=2 if ib == BPC - 1 else 1,
                        )
                    )
            avst[ib] = (avs, w_)

        pend_copy = []

        def stage_C(ib, defer_copy=False):
            pv = pv_pool.tile([K, C + 1], F32, tag="pv")  # [vlad_raw | -asum]
            avs, w_ = avst[ib]
            for ch in range(CHUNKS):
                nc.tensor.matmul(
                    pv,
                    lhsT=avs[ch // w_][:, :, ch % w_],
                    rhs=x2t[ib][ch // GROUP][:, ch % GROUP, :],
                    start=(ch == 0),
                    stop=(ch == CHUNKS - 1),
                )
            if defer_copy:
                pend_copy.append(pv)
            else:
                flush_copies()
                outt = opool.tile([K, C + 1], F32)
                if ib == BPC - 1:
                    # last batch: DVE is free after the final scale and its
                    # PSUM copy is slightly cheaper than ACT's
                    nc.vector.tensor_copy(out=outt, in_=pv)
                else:
                    nc.scalar.copy(out=outt, in_=pv)
                outts.append(outt)

        def flush_copies():
            while pend_copy:
                pvp = pend_copy.pop(0)
                outt = opool.tile([K, C + 1], F32)
                nc.scalar.copy(out=outt, in_=pvp)
                outts.append(outt)


        # PE p-state warmup: ~3.3us of dummy matmuls on const data ramp the
        # tensor engine to full clock before the first real mm1s issue.
        warm_a = const.tile([NCHUNK, K], BF16)
        warm_b = const.tile([NCHUNK, C + 1], BF16)
        nc.vector.memset(warm_a, 0.0)
        nc.vector.memset(warm_b, 0.0)
        wpv = pv_pool.tile([K, C + 1], F32, tag="pv")
        NWARM = 16
        for i in range(NWARM):
            nc.tensor.matmul(
                wpv, lhsT=warm_a, rhs=warm_b, start=(i == 0), stop=(i == NWARM - 1)
            )
        warm_out = const.tile([K, 1], F32)
        nc.vector.tensor_copy(out=warm_out, in_=wpv[:, 0:1])  # releases wpv

        # DMA stream: x1 loads lead their batch's x2 by ~2 slots so each
        # softmax chain (exp-paced) finishes just as its x2 lands; late x2s
        # then gate only PE mm2 work.  Tuned against the timeline model.
        stream = [
            ("x1", 0), ("w", None), ("x1", 1), ("x2", 0), ("x1", 2), ("x1", 3),
            ("x2", 1), ("x1", 4), ("x1", 5), ("x2", 2), ("x1", 6),
            ("x2", 3), ("x1", 7), ("x2", 4), ("x2", 5), ("x2", 6), ("x2", 7),
        ]
        for kind, ib in stream:
            if kind == "w":
                # GPSIMD SWDGE queue: doesn't displace the x stream on the SP
                # HWDGE queue, lands before mm1[0] needs it
                nc.gpsimd.dma_start(out=w_sb, in_=w_d)
            else:
                (load_x1 if kind == "x1" else load_x2)(ib)

        # compute issue order follows the stream's arrival order: A[b] right
        # after its x1 slot, C[b] one A-slot after its x2 slot (so a
        # softmax-gated C never head-of-line-blocks a ready A in the in-order
        # PE queue).
        for kind, b in [("A", 0), ("A", 1), ("A", 2), ("A", 3), ("C", 0),
                        ("A", 4), ("C", 1), ("A", 5), ("C", 2), ("A", 6),
                        ("C", 3), ("A", 7), ("C", 4), ("C", 5), ("C", 6), ("C", 7)]:
            if kind == "A":
                stage_AB(b)
            else:
                # copies of mid-stream batches would interleave the (saturated)
                # ACT exp queue; defer them until after the last exp issues
                stage_C(b)

        # all output DMAs after the x loads so they never head-of-line block
        # the (bottleneck) input stream on the sync queue
        for ib, outt in enumerate(outts):
            if outt is not None:
                nc.sync.dma_start(out=out_d[ib], in_=outt)


_NC_CACHE = None


def _get_nc():
    global _NC_CACHE
    if _NC_CACHE is None:
        nc = bacc.Bacc(
            "TRN2",
            target_bir_lowering=False,
            debug=False,
            num_devices=NCORES,
        )
        x1_d = nc.dram_tensor("x1", [BPC, C, HW], FP8, kind="ExternalInput").ap()
        x2_d = nc.dram_tensor(
            "x2", [BPC, NCHUNK, CHUNKS, C + 1], FP8, kind="ExternalInput"
        ).ap()
        w_d = nc.dram_tensor("w_t", [C, K], BF16, kind="ExternalInput").ap()
        out_d = nc.dram_tensor("out", [BPC, K, C + 1], F32, kind="ExternalOutput").ap()
        with tile.TileContext(nc) as tc:
            _netvlad_tile(tc, out_d, x1_d, x2_d, w_d)
        nc.compile()
        _NC_CACHE = nc
    return _NC_CACHE


def _make_in_maps(x, conv_w):
    bf16 = ml_dtypes.bfloat16
    f8 = ml_dtypes.float8_e4m3fn
    x1 = np.ascontiguousarray(x.reshape(B, C, HW)).astype(f8)  # [B, C, N]
    # [B, n%128, n//128, C] so mm2's rhs tiles DMA as contiguous rows
    xt = np.ascontiguousarray(
        x1.reshape(B, C, CHUNKS, NCHUNK).transpose(0, 3, 2, 1)
    )
    x2 = np.empty((B, NCHUNK, CHUNKS, C + 1), dtype=f8)
    x2[..., :C] = xt
    x2[..., C] = -1.0
    w_t = np.ascontiguousarray(conv_w.T.astype(bf16))  # [C, K]
    in_maps = []
    for core in range(NCORES):
        sl = slice(core * BPC, (core + 1) * BPC)
        in_maps.append({"x1": x1[sl], "x2": x2[sl], "w_t": w_t})
    return in_maps


def _run(in_maps, trace=False, **kwargs):
    nc = _get_nc()
    return bass_utils.run_bass_kernel_spmd(
        nc, in_maps, core_ids=list(range(NCORES)), trace=trace, **kwargs
    )


def _postprocess(raw, centroids):
    """raw: [B, K, C+1] = [vlad_raw | -asum]  ->  [B, K*C] normalized."""
    vlad = raw[:, :, :C] + raw[:, :, C : C + 1] * centroids[None, :, :]
    norms = np.sqrt((vlad * vlad).sum(axis=2, keepdims=True))
    vlad = vlad / np.maximum(norms, 1e-12)
    out = vlad.reshape(raw.shape[0], K * C)
    gn = np.sqrt((out * out).sum(axis=1, keepdims=True))
    return out / np.maximum(gn, 1e-12)


def kernel(x, conv_w, centroids):
    x = np.asarray(x)
    conv_w = np.asarray(conv_w)
    centroids = np.asarray(centroids, dtype=np.float32)
    res = _run(_make_in_maps(x, conv_w))
    raw = np.concatenate([r["out"] for r in res.results], axis=0)  # [B, K, C+1]
    return _postprocess(raw.astype(np.float32), centroids).astype(np.float32)
